# revision 1
# baseline (speedup 1.0000x reference)
"""Trainium2 Bass kernel for nn_CaduceusEmbeddingsSTFT.

out[b, t, :] = concat(emb_table[ids[b, t]],
                      proj(|STFT(onehot(ids[b]))| upsampled at frame f(t)))

Structure exploited:
  * nearest upsampling -> only 129 distinct STFT frame rows per batch; the
    (8192 x 2064) @ (2064 x 154) projection collapses to (129 x 2064) @
    (2064 x 154) plus a row broadcast.
  * STFT of one-hot signals: windowed frames are one-hot masks, so
    spec = onehot_frames @ (window * DFT) as matmuls (cos / sin).
  * embedding lookup and frame broadcast are one-hot matmuls on the PE.

All matmuls run in bf16 (fp32 matmuls execute as two passes and tend to
stay HAM-throttled). Precision is preserved by exact hi+lo bf16 splits:
one-hot operands are exact in bf16; the other side is split so
x = hi + lo with both parts bf16 and the product accumulated in fp32
PSUM (omitted lo*lo cross terms are ~2^-18 relative).

Sharding: 8 cores = 4 batches x 2 sequence halves; each core computes a
(4096, 512) output shard; boundary frame recomputed by both halves.

Measured (NTFF, per core): ~57us total = ~7us NEFF/Tile preamble
+ ~5us loads/one-hots + ~38us tensor-engine stream (PE 100% occupied,
mostly at the HAM-throttled 1.2 GHz clock; K=8/8 only held during the
K=128 DFT/projection phase) + ~7us drain/exit barrier. Remaining levers,
in value order: (1) defeat the mid-kernel HAM re-throttle (zero-padding
the small-K output matmuls to K=128 did NOT release it; suspect P0
power-state downclock per engine docs); (2) PSUM->SBUF drain is
~23us each on DVE+ACT and paces the output phase (DMA cannot read PSUM,
so every output byte crosses one of them); (3) the Tile entry/exit
barriers (~14us combined) are fixed per-kernel cost under TileContext.
"""

import numpy as np

V = 16
D_EMB = 358
D_STFT = 154
NFFT = 256
HOP = 64
NFREQ = 129
B, L = 4, 8192
LH = L // 2  # 4096 rows per core
F = 65  # frames per core (inclusive overlap frame)
VF = V * F  # 1040
DM = 512
NCORES = 8
NT = LH // 128  # 32 output tiles per core
NQ = NT // 4  # q-groups of 4 tiles
# (start, size) chunks over the VF axis; multiples of F so projection
# lhsT slices [:, v*F:(v+1)*F] never cross a chunk boundary.
CHUNKS = [(0, 7 * F), (7 * F, 7 * F), (14 * F, 2 * F)]

_PROG = None
LAST_RESULT = None  # BassKernelResults of the most recent run (for harnesses)


def _build_program():
    import concourse.mybir as mybir
    import concourse.tile as tile
    from concourse import bacc

    f32 = mybir.dt.float32
    bf16 = mybir.dt.bfloat16
    i8 = mybir.dt.int8
    AO = mybir.AluOpType
    AF = mybir.ActivationFunctionType

    nc = bacc.Bacc("TRN2", target_bir_lowering=False, debug=False,
                   num_devices=NCORES)

    CWW = 2 * NFREQ  # 258: per-(c,part) block width in cw
    # packed int8 ids block: [h_emb 1024 | vemb 1 | h_frames 130 | vfr 16]
    IPW = LH // 4 + 1 + 2 * F + V
    # packed bf16 consts: [embrep 358 | cw 4*258 | wnyq 2*154 (rows 0..V)]
    APW = D_EMB + 4 * CWW + 2 * D_STFT

    ipack = nc.dram_tensor("ipack", [128, IPW], i8, kind="ExternalInput")
    apack = nc.dram_tensor("apack", [128, APW], bf16, kind="ExternalInput")
    bsel = nc.dram_tensor("bsel", [128, LH], bf16, kind="ExternalInput")
    wproj = nc.dram_tensor("wproj", [128, 2 * V * D_STFT], bf16,
                           kind="ExternalInput")
    out = nc.dram_tensor("out", [LH, DM], f32, kind="ExternalOutput")

    with tile.TileContext(nc) as tc:
        with (
            tc.tile_pool(name="consts", bufs=1) as cpool,
            tc.tile_pool(name="work", bufs=1) as wpool,
            tc.tile_pool(name="tmp", bufs=2) as tpool,
            tc.tile_pool(name="oemb", bufs=3) as oepool,
            tc.tile_pool(name="ostft", bufs=3) as ospool,
        ):
            # ---- const loads: 4 packed DMAs (Sync issue cost ~0.6us each) ---
            IP = cpool.tile([128, IPW], i8, tag="ip")
            nc.sync.dma_start(out=IP[:], in_=ipack[:])
            AP_ = cpool.tile([128, APW], bf16, tag="ap")
            nc.sync.dma_start(out=AP_[:], in_=apack[:])
            BS = cpool.tile([128, LH], bf16, tag="bs")
            nc.sync.dma_start(out=BS[:], in_=bsel[:])
            WP = cpool.tile([128, 2 * V * D_STFT], bf16, tag="wp")
            nc.sync.dma_start(out=WP[:], in_=wproj[:])

            HE = IP[:, :LH // 4]
            VEMB = IP[:, LH // 4:LH // 4 + 1]
            HF = IP[:, LH // 4 + 1:LH // 4 + 1 + 2 * F]
            VFR = IP[:, LH // 4 + 1 + 2 * F:]
            ER = AP_[:, :D_EMB]
            CW = AP_[:, D_EMB:D_EMB + 4 * CWW]
            WN = AP_[0:V + 1, D_EMB + 4 * CWW:]

            # ---- one-hot builds (bf16 out: 0/1 exact) -----------------------
            OHE = wpool.tile([128, LH // 4], bf16, tag="ohe")
            nc.vector.tensor_tensor(
                out=OHE[:], in0=HE, in1=VEMB.to_broadcast([128, LH // 4]),
                op=AO.is_equal)
            OHF = []
            for c in range(2):
                t = wpool.tile([128, VF], bf16, tag=f"ohf{c}")
                in0 = (HF[:, c * F:(c + 1) * F]
                       .rearrange("p (one f) -> p one f", one=1)
                       .to_broadcast([128, V, F]))
                in1 = (VFR.rearrange("p (v one) -> p v one", one=1)
                       .to_broadcast([128, V, F]))
                nc.vector.tensor_tensor(
                    out=t[:].rearrange("p (v f) -> p v f", v=V),
                    in0=in0, in1=in1, op=AO.is_equal)
                OHF.append(t)

            MAG = wpool.tile([128, VF], f32, tag="mag")
            MAGH = wpool.tile([128, VF], bf16, tag="magh")
            MAGL = wpool.tile([128, VF], bf16, tag="magl")
            MAGN = wpool.tile([1, VF], f32, tag="magn")

            with (
                tc.tile_pool(name="psum_re", bufs=1, space="PSUM") as pre,
                tc.tile_pool(name="psum_im", bufs=1, space="PSUM") as pim,
                tc.tile_pool(name="psum_ny", bufs=1, space="PSUM") as pny,
                tc.tile_pool(name="psum_emb", bufs=4, space="PSUM") as pemb,
                tc.tile_pool(name="psum_s", bufs=1, space="PSUM") as psp,
            ):
                # ---- emb pipeline: starts as soon as HE/VEMB/ER land --------
                for q in range(NQ):
                    oe = oepool.tile([128, 4 * D_EMB], f32, tag="oe")
                    for a in range(4):
                        po = pemb.tile([128, D_EMB], f32, tag="pe")
                        nc.tensor.matmul(
                            out=po[:],
                            lhsT=OHE[32 * a:32 * (a + 1),
                                     q * 128:(q + 1) * 128],
                            rhs=ER[32 * a:32 * (a + 1), :],
                            start=True, stop=True,
                            tile_position=(32 * a, 0))
                        sl = oe[:, a * D_EMB:(a + 1) * D_EMB]
                        if a % 2 == 0:
                            nc.vector.tensor_copy(out=sl, in_=po[:])
                        else:
                            nc.scalar.copy(out=sl, in_=po[:])
                    nc.sync.dma_start(
                        out=out[q * 512:(q + 1) * 512, :D_EMB]
                        .rearrange("(a p) e -> p a e", p=128),
                        in_=oe[:].rearrange("p (a e) -> p a e", a=4))

                # ---- DFT + mag + projection (high priority: S gates the
                # stft half of every output tile) ------------------------------
                S = psp.tile([F, D_STFT], f32, tag="s")
                first_s = [True]

                def proj_mm(lhsT, rhs, stop=False):
                    nc.tensor.matmul(out=S[:], lhsT=lhsT, rhs=rhs,
                                     start=first_s[0], stop=stop)
                    first_s[0] = False

                for ci, (c0, cn) in enumerate(CHUNKS):
                    re = pre.tile([128, cn], f32, tag="re")
                    im = pim.tile([128, cn], f32, tag="im")
                    ny = pny.tile([1, cn], f32, tag="ny")
                    first = True
                    for c in range(2):
                        for part in range(2):  # hi, lo
                            cb = (2 * c + part) * CWW
                            rhs = OHF[c][:, c0:c0 + cn]
                            nc.tensor.matmul(
                                out=re[:], lhsT=CW[:, cb:cb + 128], rhs=rhs,
                                start=first, stop=(c == 1 and part == 1))
                            nc.tensor.matmul(
                                out=im[:],
                                lhsT=CW[:, cb + NFREQ:cb + NFREQ + 128],
                                rhs=rhs,
                                start=first, stop=(c == 1 and part == 1))
                            nc.tensor.matmul(
                                out=ny[:], lhsT=CW[:, cb + 128:cb + 129],
                                rhs=rhs,
                                start=(c == 0 and part == 0),
                                stop=(c == 1 and part == 1))
                            first = False
                    with tc.high_priority():
                        # |spec| = sqrt(re^2 + im^2); squares on ACT (DVE
                        # cannot read two PSUM operands)
                        t1 = tpool.tile([128, cn], f32, tag="sq1")
                        t2 = tpool.tile([128, cn], f32, tag="sq2")
                        nc.scalar.square(out=t1[:], in_=re[:])
                        nc.scalar.square(out=t2[:], in_=im[:])
                        nc.vector.tensor_tensor(out=t1[:], in0=t1[:],
                                                in1=t2[:], op=AO.add)
                        nc.scalar.sqrt(out=MAG[:, c0:c0 + cn], in_=t1[:])
                        nc.scalar.activation(MAGN[:, c0:c0 + cn], ny[:],
                                             AF.Abs)
                        # per-chunk bf16 hi/lo split of MAG
                        nc.vector.tensor_copy(out=MAGH[:, c0:c0 + cn],
                                              in_=MAG[:, c0:c0 + cn])
                        nc.vector.tensor_tensor(
                            out=MAGL[:, c0:c0 + cn], in0=MAG[:, c0:c0 + cn],
                            in1=MAGH[:, c0:c0 + cn], op=AO.subtract)
                        # projection matmuls for this chunk's v range
                        for v in range(c0 // F, (c0 + cn) // F):
                            mh = MAGH[:, v * F:(v + 1) * F]
                            ml = MAGL[:, v * F:(v + 1) * F]
                            wh = WP[:, (2 * v) * D_STFT:(2 * v + 1) * D_STFT]
                            wl = WP[:, (2 * v + 1) * D_STFT:
                                    (2 * v + 2) * D_STFT]
                            proj_mm(mh, wh)
                            proj_mm(mh, wl)
                            proj_mm(ml, wh)

                with tc.high_priority():
                    # nyquist row -> (V, F) fold; extra ones/zeros column
                    # block becomes the K=V+1 bias row (WN row V = proj_b)
                    NYH = wpool.tile([1, VF + F], bf16, tag="nyh")
                    NYL = wpool.tile([1, VF + F], bf16, tag="nyl")
                    nc.vector.tensor_copy(out=NYH[:, :VF], in_=MAGN[:, :])
                    nc.vector.tensor_tensor(out=NYL[:, :VF], in0=MAGN[:, :],
                                            in1=NYH[:, :VF], op=AO.subtract)
                    nc.vector.memset(NYH[:, VF:], 1.0)
                    nc.vector.memset(NYL[:, VF:], 0.0)
                    NYQTH = wpool.tile([V + 1, F], bf16, tag="nyqth")
                    NYQTL = wpool.tile([V + 1, F], bf16, tag="nyqtl")
                    nc.sync.dma_start(out=NYQTH[:, :], in_=NYH[:, :])
                    nc.sync.dma_start(out=NYQTL[:, :], in_=NYL[:, :])
                    proj_mm(NYQTH[:], WN[:, :D_STFT])
                    proj_mm(NYQTH[:], WN[:, D_STFT:])
                    proj_mm(NYQTL[:], WN[:, :D_STFT], stop=True)
                    # S split to bf16 hi/lo (bias already inside S),
                    # zero-padded to K=128 so the stft matmuls keep the PE
                    # activity monitor (HAM) warm
                    SH = wpool.tile([128, D_STFT], bf16, tag="sh")
                    SL = wpool.tile([128, D_STFT], bf16, tag="sl")
                    nc.vector.memset(SH[:], 0.0)
                    nc.vector.memset(SL[:], 0.0)
                    nc.vector.tensor_copy(out=SH[:F, :], in_=S[:])
                    nc.vector.tensor_tensor(out=SL[:F, :], in0=S[:],
                                            in1=SH[:F, :], op=AO.subtract)

            # ---- stft part of output: B-select @ (SH + SL) ------------------
            # two tiles share one PSUM bank (2*154*4B < 2KB): halves the
            # PSUM->SBUF copy op count
            with tc.tile_pool(name="psum_stft", bufs=4, space="PSUM") as pstft:
                for q in range(NQ):
                    os_ = ospool.tile([128, 4 * D_STFT], f32, tag="os")
                    for half in range(2):
                        ps = pstft.tile([128, 2 * D_STFT], f32, tag="ps")
                        for sub in range(2):
                            a = half * 2 + sub
                            ti = q * 4 + a
                            lhsT = BS[:, ti * 128:(ti + 1) * 128]
                            po = ps[:, sub * D_STFT:(sub + 1) * D_STFT]
                            nc.tensor.matmul(out=po, lhsT=lhsT, rhs=SH[:],
                                             start=True, stop=False)
                            nc.tensor.matmul(out=po, lhsT=lhsT, rhs=SL[:],
                                             start=False, stop=True)
                        sl = os_[:, half * 2 * D_STFT:(half + 1) * 2 * D_STFT]
                        if (q + half) % 2 == 0:
                            nc.scalar.copy(out=sl, in_=ps[:])
                        else:
                            nc.vector.tensor_copy(out=sl, in_=ps[:])
                    nc.sync.dma_start(
                        out=out[q * 512:(q + 1) * 512, D_EMB:DM]
                        .rearrange("(a p) e -> p a e", p=128),
                        in_=os_[:].rearrange("p (a e) -> p a e", a=4))

    nc.finalize()
    return nc


def _split_bf16(x):
    import ml_dtypes

    hi = x.astype(ml_dtypes.bfloat16)
    lo = (x - hi.astype(np.float32)).astype(ml_dtypes.bfloat16)
    return hi, lo


def _host_consts():
    import ml_dtypes

    bf16 = ml_dtypes.bfloat16
    n = np.arange(NFFT)
    window = 0.5 - 0.5 * np.cos(2.0 * np.pi * n / NFFT)
    k = np.arange(NFREQ)
    ang = 2.0 * np.pi * np.outer(n, k) / NFFT  # (256, 129)
    wcos = (window[:, None] * np.cos(ang)).astype(np.float32)
    wsin = (window[:, None] * np.sin(ang)).astype(np.float32)
    CWW = 2 * NFREQ
    cwf = np.zeros((128, 4 * CWW), np.float32)
    for c in range(2):
        rows = slice(c * 128, (c + 1) * 128)
        # block layout per (c, part): [cos k0..k127, cos k128, sin k0..k127, 0]
        blk = np.zeros((128, CWW), np.float32)
        blk[:, :128] = wcos[rows, :128]
        blk[:, 128] = wcos[rows][:, 128]  # nyquist cos column
        blk[:, NFREQ:NFREQ + 128] = wsin[rows, :128]
        hi, lo = _split_bf16(blk)
        cwf[:, (2 * c) * CWW:(2 * c + 1) * CWW] = hi.astype(np.float32)
        cwf[:, (2 * c + 1) * CWW:(2 * c + 2) * CWW] = lo.astype(np.float32)
    cw = cwf.astype(bf16)

    vfr = np.broadcast_to(np.arange(V, dtype=np.int8), (128, V)).copy()
    vemb = (np.arange(128, dtype=np.int8) % 32).reshape(128, 1).copy()
    return cw, vfr, vemb


def _bsel_for_half(h):
    import ml_dtypes

    t = np.arange(LH)
    fglob = (129 * (t + LH * h)) >> 13
    floc = fglob - 64 * h
    bs = np.zeros((128, LH), np.float32)
    bs[floc, t] = 1.0
    return bs.astype(ml_dtypes.bfloat16)


def kernel(input_ids, emb_table, proj_w, proj_b):
    global _PROG, LAST_RESULT
    import ml_dtypes

    from concourse.bass_utils import run_bass_kernel_spmd

    bf16 = ml_dtypes.bfloat16
    ids = np.asarray(input_ids).astype(np.int32)
    emb = np.asarray(emb_table).astype(np.float32)
    pw = np.asarray(proj_w).astype(np.float32)
    pb = np.asarray(proj_b).astype(np.float32)

    cw, vfr, vemb = _host_consts()

    # proj_w rows are indexed by i = k*V + v (freq-major); interleave hi/lo
    wproj = np.zeros((128, 2 * V * D_STFT), np.float32)
    for v in range(V):
        hi, lo = _split_bf16(pw[np.arange(128) * V + v])
        wproj[:, (2 * v) * D_STFT:(2 * v + 1) * D_STFT] = hi.astype(np.float32)
        wproj[:, (2 * v + 1) * D_STFT:(2 * v + 2) * D_STFT] = \
            lo.astype(np.float32)
    wproj = wproj.astype(bf16)
    # nyquist proj weights + bias row (fed by the ones column of NYQTH)
    nh, nl = _split_bf16(pw[128 * V + np.arange(V)])
    bh, bl = _split_bf16(pb.reshape(1, D_STFT))
    wnyq = np.zeros((V + 1, 2 * D_STFT), np.float32)
    wnyq[:V, :D_STFT] = nh.astype(np.float32)
    wnyq[:V, D_STFT:] = nl.astype(np.float32)
    wnyq[V, :D_STFT] = bh.astype(np.float32)
    wnyq[V, D_STFT:] = bl.astype(np.float32)
    wnyq = wnyq.astype(bf16)

    embrep = np.zeros((128, D_EMB), np.float32)
    for a in range(4):
        embrep[32 * a:32 * a + V] = emb
    embrep = embrep.astype(bf16)

    # apack: [embrep | cw | wnyq rows 0..V] (bf16)
    apack = np.zeros((128, D_EMB + cw.shape[1] + 2 * D_STFT), bf16)
    apack[:, :D_EMB] = embrep
    apack[:, D_EMB:D_EMB + cw.shape[1]] = cw
    apack[:V + 1, D_EMB + cw.shape[1]:] = wnyq

    bsel = [_bsel_for_half(h) for h in range(2)]

    in_maps = []
    for core in range(NCORES):
        b, h = divmod(core, 2)
        padded = np.pad(ids[b], 128, mode="reflect")
        seg = padded[LH * h:LH * h + 64 * (F - 1) + NFFT]  # (4352,)
        hf = np.zeros((128, 2 * F), np.int8)
        for c in range(2):
            idx = (64 * np.arange(F)[None, :] + 128 * c
                   + np.arange(128)[:, None])
            hf[:, c * F:(c + 1) * F] = seg[idx]
        ids_out = ids[b, LH * h:LH * (h + 1)]
        he = np.zeros((128, LH // 4), np.int8)
        tiles = ids_out.reshape(NT, 128)  # tile ti = 4q+a
        for a in range(4):
            rows = tiles[a::4]  # (8, 128), q-major
            he[32 * a:32 * a + V, :] = np.broadcast_to(
                rows.reshape(1, LH // 4), (V, LH // 4))
        # ipack: [h_emb | vemb | h_frames | vfr] (int8)
        ipack = np.concatenate([he, vemb, hf, vfr], axis=1)
        in_maps.append({
            "ipack": ipack, "apack": apack, "bsel": bsel[h], "wproj": wproj,
        })

    if _PROG is None:
        _PROG = _build_program()

    res = run_bass_kernel_spmd(_PROG, in_maps, core_ids=list(range(NCORES)))
    LAST_RESULT = res

    full = np.zeros((B, L, DM), np.float32)
    for core in range(NCORES):
        b, h = divmod(core, 2)
        full[b, LH * h:LH * (h + 1), :] = res.results[core]["out"]
    return full



# revision 12
# speedup vs baseline: 1.3480x; 1.3480x over previous
"""Trainium2 Bass kernel for nn_CaduceusEmbeddingsSTFT.

out[b, t, :] = concat(emb_table[ids[b, t]],
                      proj(|STFT(onehot(ids[b]))| upsampled at frame f(t)))

Structure exploited:
  * nearest upsampling -> only 65 distinct STFT frame rows per core-half;
    the projection collapses to (65 x 2064) @ (2064 x 154).
  * STFT of one-hot signals: windowed frames are one-hot masks, so
    spec = onehot_frames @ (window * DFT) as matmuls (cos / sin).
  * every output row is concat(emb_row[id(t)], S[frame(t)]) -- built by a
    SINGLE matmul per 128-row tile: lhsT stacks the id one-hot (16 rows)
    and the frame one-hot (65 rows), rhs is the table [emb | S].

Precision: harness gate is rel_err < 2e-2; everything runs plain bf16
(fp32 PSUM accumulation), output DMA'd as bf16 and upcast on host.
Measured numpy sim of this scheme: rel err ~4.8e-3.

Sharding: 8 cores = 4 batches x 2 sequence halves; each core computes a
(4096, 512) output shard; boundary frame recomputed by both halves.

Perf design (per core): ~18 dummy matmuls pre-warm the PE HAM clock
while input DMAs land; DFT (6 streams x 1040 cols) -> |mag| on ACT/DVE
-> projection (16 K=128 MMs + 16 K=1 nyquist MMs, bias via DVE
broadcast-add into the rhs table) -> 32 fused N=512 output MMs; drains
alternate DVE/ACT; output leaves as bf16 (4 MB/core, ~12 us DMA).
"""

import numpy as np

V = 16
D_EMB = 358
D_STFT = 154
NFFT = 256
HOP = 64
NFREQ = 129
B, L = 4, 8192
LH = L // 2  # 4096 rows per core
F = 65  # frames per core (inclusive overlap frame)
VF = V * F  # 1040
DM = 512
NCORES = 8
NT = LH // 128  # 32 output tiles per core
NQ = NT // 4  # q-groups of 4 tiles
KOUT = V + F  # 81: stacked one-hot rows in the output matmul
CWW = 2 * NFREQ  # 258: per-c block width in cw (cos 0..127 | ny | sin 0..127)
# (start, size) chunks over the VF axis; multiples of F so projection
# lhsT slices [:, v*F:(v+1)*F] never cross a chunk boundary; <=512 f32
# per PSUM bank.
CHUNKS = [(0, 7 * F), (7 * F, 7 * F), (14 * F, 2 * F)]
NDUM = 18  # PE warm-up matmuls issued while input DMAs land

PK1W = 2 * VF + 2 * CWW  # [ohf0 | ohf1 | cw]
BFW = LH + DM  # [one-hot select cols | rhs-table init image]

_PROG = None
LAST_RESULT = None  # BassKernelResults of the most recent run (for harnesses)


def _build_program():
    import concourse.mybir as mybir
    import concourse.tile as tile
    from concourse import bacc

    f32 = mybir.dt.float32
    bf16 = mybir.dt.bfloat16
    AO = mybir.AluOpType
    AF = mybir.ActivationFunctionType

    nc = bacc.Bacc("TRN2", target_bir_lowering=False, debug=False,
                   num_devices=NCORES)

    pk1 = nc.dram_tensor("pk1", [128, PK1W], bf16, kind="ExternalInput")
    pk2 = nc.dram_tensor("pk2", [128, V * D_STFT], bf16, kind="ExternalInput")
    nyw = nc.dram_tensor("nyw", [1, V * D_STFT], bf16, kind="ExternalInput")
    bfsel = nc.dram_tensor("bfsel", [KOUT, BFW], bf16, kind="ExternalInput")
    out = nc.dram_tensor("out", [LH, DM], bf16, kind="ExternalOutput")

    with tile.TileContext(nc) as tc:
        with (
            tc.tile_pool(name="consts", bufs=1) as cpool,
            tc.tile_pool(name="work", bufs=1) as wpool,
            tc.tile_pool(name="tmp", bufs=2) as tpool,
            tc.tile_pool(name="ostg", bufs=3) as ospool,
        ):
            # ---- const loads (issue order = need order) ---------------------
            PK1 = cpool.tile([128, PK1W], bf16, tag="pk1")
            nc.sync.dma_start(out=PK1[:], in_=pk1[:])
            PK2 = cpool.tile([128, V * D_STFT], bf16, tag="pk2")
            nc.sync.dma_start(out=PK2[:], in_=pk2[:])
            BF = cpool.tile([KOUT, BFW], bf16, tag="bf")
            nc.sync.dma_start(out=BF[:], in_=bfsel[:])
            NYW = cpool.tile([1, V * D_STFT], bf16, tag="nyw")
            nc.sync.dma_start(out=NYW[:], in_=nyw[:])

            OHF = [PK1[:, 0:VF], PK1[:, VF:2 * VF]]
            CW = PK1[:, 2 * VF:]
            WP = PK2
            WNR = NYW

            # ---- on-chip work tiles ----------------------------------------
            ZW = wpool.tile([128, 128], bf16, tag="zw")
            nc.vector.memset(ZW[:], 0.0)
            # rhs table init: rows 0..64 [0 | bias], rows 65..80 [emb | 0]
            RT = wpool.tile([KOUT, DM], bf16, tag="rt")
            nc.vector.tensor_copy(out=RT[:], in_=BF[:, LH:])
            MAGH = wpool.tile([128, VF], bf16, tag="magh")
            NYB = wpool.tile([1, VF], bf16, tag="nyb")

            with tc.tile_pool(name="psum_s", bufs=1, space="PSUM") as psp:
                S = psp.tile([F, D_STFT], f32, tag="s")

                with (
                    tc.tile_pool(name="psum_dum", bufs=1, space="PSUM") as pdm,
                    tc.tile_pool(name="psum_re", bufs=2, space="PSUM") as pre,
                    tc.tile_pool(name="psum_im", bufs=2, space="PSUM") as pim,
                    tc.tile_pool(name="psum_ny", bufs=2, space="PSUM") as pny,
                ):
                    # PE warm-up: no input deps, scheduler runs these first;
                    # ~18 x (ldw+mm) ~= 3.4us busy -> HAM releases to 2.4 GHz
                    # right as the first real matmul's data lands.
                    DU = pdm.tile([128, 128], f32, tag="du")
                    for _ in range(NDUM):
                        nc.tensor.matmul(out=DU[:], lhsT=ZW[:], rhs=ZW[:],
                                         start=True, stop=True)

                    first_s = [True]

                    def proj_mm(lhsT, rhs, stop=False):
                        nc.tensor.matmul(out=S[:], lhsT=lhsT, rhs=rhs,
                                         start=first_s[0], stop=stop)
                        first_s[0] = False

                    for c0, cn in CHUNKS:
                        re = pre.tile([128, 7 * F], f32, tag="re")
                        im = pim.tile([128, 7 * F], f32, tag="im")
                        ny = pny.tile([1, 7 * F], f32, tag="ny")
                        for c in range(2):
                            cb = c * CWW
                            rhs = OHF[c][:, c0:c0 + cn]
                            nc.tensor.matmul(
                                out=re[:, :cn], lhsT=CW[:, cb:cb + 128],
                                rhs=rhs, start=(c == 0), stop=(c == 1))
                            nc.tensor.matmul(
                                out=im[:, :cn],
                                lhsT=CW[:, cb + NFREQ:cb + NFREQ + 128],
                                rhs=rhs, start=(c == 0), stop=(c == 1))
                            nc.tensor.matmul(
                                out=ny[:, :cn], lhsT=CW[:, cb + 128:cb + 129],
                                rhs=rhs, start=(c == 0), stop=(c == 1))
                        with tc.high_priority():
                            # |spec| = sqrt(re^2 + im^2); squares on ACT (DVE
                            # cannot read two PSUM operands)
                            t1 = tpool.tile([128, 7 * F], f32, tag="sq1")
                            t2 = tpool.tile([128, 7 * F], f32, tag="sq2")
                            nc.scalar.square(out=t1[:, :cn], in_=re[:, :cn])
                            nc.scalar.square(out=t2[:, :cn], in_=im[:, :cn])
                            nc.vector.tensor_tensor(
                                out=t1[:, :cn], in0=t1[:, :cn], in1=t2[:, :cn],
                                op=AO.add)
                            nc.scalar.sqrt(out=MAGH[:, c0:c0 + cn],
                                           in_=t1[:, :cn])
                            nc.scalar.activation(NYB[:, c0:c0 + cn],
                                                 ny[:, :cn], AF.Abs)
                            for v in range(c0 // F, (c0 + cn) // F):
                                proj_mm(MAGH[:, v * F:(v + 1) * F],
                                        WP[:, v * D_STFT:(v + 1) * D_STFT])

                    with tc.high_priority():
                        # nyquist bin: 16 tiny K=1 matmuls off the |ny| row
                        for v in range(V):
                            proj_mm(NYB[0:1, v * F:(v + 1) * F],
                                    WNR[0:1, v * D_STFT:(v + 1) * D_STFT],
                                    stop=(v == V - 1))

                with tc.high_priority():
                    # rhs table rows 0..64 = S + bias image (in-place add)
                    nc.vector.tensor_tensor(
                        out=RT[0:F, D_EMB:DM], in0=S[:],
                        in1=RT[0:F, D_EMB:DM], op=AO.add)

            # ---- output: one fused matmul per 128-row tile ------------------
            with tc.tile_pool(name="psum_out", bufs=7, space="PSUM") as pout:
                for q in range(NQ):
                    os_ = ospool.tile([128, 4 * DM], bf16, tag="os")
                    for a in range(4):
                        ti = q * 4 + a
                        po = pout.tile([128, DM], f32, tag="po")
                        nc.tensor.matmul(
                            out=po[:], lhsT=BF[:, ti * 128:(ti + 1) * 128],
                            rhs=RT[0:KOUT, :], start=True, stop=True)
                        sl = os_[:, a * DM:(a + 1) * DM]
                        if a % 2 == 0:
                            nc.vector.tensor_copy(out=sl, in_=po[:])
                        else:
                            nc.scalar.copy(out=sl, in_=po[:])
                    nc.sync.dma_start(
                        out=out[q * 512:(q + 1) * 512, :]
                        .rearrange("(a p) e -> p a e", p=128),
                        in_=os_[:].rearrange("p (a e) -> p a e", a=4))

    nc.finalize()
    return nc


def _host_consts():
    import ml_dtypes

    bf16 = ml_dtypes.bfloat16
    n = np.arange(NFFT)
    window = 0.5 - 0.5 * np.cos(2.0 * np.pi * n / NFFT)
    k = np.arange(NFREQ)
    ang = 2.0 * np.pi * np.outer(n, k) / NFFT  # (256, 129)
    wcos = (window[:, None] * np.cos(ang)).astype(np.float32)
    wsin = (window[:, None] * np.sin(ang)).astype(np.float32)
    cw = np.zeros((128, 2 * CWW), np.float32)
    for c in range(2):
        rows = slice(c * 128, (c + 1) * 128)
        blk = np.zeros((128, CWW), np.float32)
        blk[:, :128] = wcos[rows, :128]
        blk[:, 128] = wcos[rows][:, 128]  # nyquist cos column
        blk[:, NFREQ:NFREQ + 128] = wsin[rows, :128]
        cw[:, c * CWW:(c + 1) * CWW] = blk
    return cw.astype(bf16)


def kernel(input_ids, emb_table, proj_w, proj_b):
    global _PROG, LAST_RESULT
    import ml_dtypes

    from concourse.bass_utils import run_bass_kernel_spmd

    bf16 = ml_dtypes.bfloat16
    ids = np.asarray(input_ids).astype(np.int64)
    emb = np.asarray(emb_table).astype(np.float32)
    pw = np.asarray(proj_w).astype(np.float32)
    pb = np.asarray(proj_b).astype(np.float32)

    cw = _host_consts()

    # pk1 cols [2*VF:] = cw; per-core ohf fills cols [:2*VF]
    # pk2: proj weights, rows k=0..127, cols v*154+o  (proj_w row i=k*V+v)
    pk2 = np.zeros((128, V * D_STFT), np.float32)
    for v in range(V):
        pk2[:, v * D_STFT:(v + 1) * D_STFT] = pw[np.arange(128) * V + v]
    pk2 = pk2.astype(bf16)

    # nyw: nyquist-bin proj weights as a single partition-0 row
    nywr = np.zeros((1, V * D_STFT), np.float32)
    for v in range(V):
        nywr[0, v * D_STFT:(v + 1) * D_STFT] = pw[128 * V + v]
    nywr = nywr.astype(bf16)

    # rhs-table init image: rows 0..64 [0 | bias], rows 65..80 [emb | 0]
    rtimg = np.zeros((KOUT, DM), np.float32)
    rtimg[:F, D_EMB:] = pb[None, :]
    rtimg[F:, :D_EMB] = emb

    vr = np.arange(V)
    in_maps = []
    for core in range(NCORES):
        b, h = divmod(core, 2)
        padded = np.pad(ids[b], 128, mode="reflect")
        seg = padded[LH * h:LH * h + 64 * (F - 1) + NFFT]  # (4352,)
        pk1 = np.zeros((128, PK1W), bf16)
        for c in range(2):
            sv = seg[(128 * c + np.arange(128))[:, None]
                     + 64 * np.arange(F)[None, :]]  # (128, F)
            oh = (sv[:, None, :] == vr[None, :, None])  # (128, V, F)
            pk1[:, c * VF:(c + 1) * VF] = oh.reshape(128, VF)
        pk1[:, 2 * VF:] = cw

        ids_h = ids[b, LH * h:LH * (h + 1)]
        t = np.arange(LH)
        floc = ((129 * (t + LH * h)) >> 13) - 64 * h
        bf = np.zeros((KOUT, BFW), np.float32)
        bf[floc, t] = 1.0
        bf[F + ids_h, t] = 1.0
        bf[:, LH:] = rtimg
        in_maps.append({
            "pk1": pk1, "pk2": pk2, "nyw": nywr,
            "bfsel": bf.astype(bf16),
        })

    if _PROG is None:
        _PROG = _build_program()

    res = run_bass_kernel_spmd(_PROG, in_maps, core_ids=list(range(NCORES)))
    LAST_RESULT = res

    full = np.zeros((B, L, DM), np.float32)
    for core in range(NCORES):
        b, h = divmod(core, 2)
        full[b, LH * h:LH * (h + 1), :] = \
            res.results[core]["out"].astype(np.float32)
    return full


# revision 15
# speedup vs baseline: 1.4617x; 1.0843x over previous
"""Trainium2 Bass kernel for nn_CaduceusEmbeddingsSTFT.

out[b, t, :] = concat(emb_table[ids[b, t]],
                      proj(|STFT(onehot(ids[b]))| upsampled at frame f(t)))

Structure exploited:
  * nearest upsampling -> only 65 distinct STFT frame rows per core-half;
    the projection collapses to (65 x 2064) @ (2064 x 154).
  * STFT of one-hot signals: windowed frames are one-hot masks, so
    spec = onehot_frames @ (window * DFT) as matmuls (cos / sin).
  * every output row is concat(emb_row[id(t)], S[frame(t)]) -- built by a
    SINGLE matmul per 128-row tile: lhsT stacks the id one-hot (16 rows)
    and the frame one-hot (65 rows), rhs is the table [emb | S].

Precision: harness gate is rel_err < 2e-2; everything runs plain bf16
(fp32 PSUM accumulation), output DMA'd as bf16 and upcast on host.
Measured numpy sim of this scheme: rel err ~4.8e-3.

Sharding: 8 cores = 4 batches x 2 sequence halves; each core computes a
(4096, 512) output shard; boundary frame recomputed by both halves.

Perf design (per core): ~18 dummy matmuls pre-warm the PE HAM clock
while input DMAs land; DFT (6 streams x 1040 cols) -> |mag| on ACT/DVE
-> projection (16 K=128 MMs + 16 K=1 nyquist MMs, bias via DVE
broadcast-add into the rhs table) -> 32 fused N=512 output MMs; drains
alternate DVE/ACT; output leaves as bf16 (4 MB/core, ~12 us DMA).
"""

import numpy as np

V = 16
D_EMB = 358
D_STFT = 154
NFFT = 256
HOP = 64
NFREQ = 129
B, L = 4, 8192
LH = L // 2  # 4096 rows per core
F = 65  # frames per core (inclusive overlap frame)
VF = V * F  # 1040
DM = 512
NCORES = 8
NT = LH // 128  # 32 output tiles per core
NQ = NT // 4  # q-groups of 4 tiles
KOUT = V + F  # 81: stacked one-hot rows in the output matmul
CWW = 2 * NFREQ  # 258: per-c block width in cw (cos 0..127 | ny | sin 0..127)
# (start, size) chunks over the VF axis; multiples of F so projection
# lhsT slices [:, v*F:(v+1)*F] never cross a chunk boundary; <=512 f32
# per PSUM bank.
CHUNKS = [(0, 7 * F), (7 * F, 7 * F), (14 * F, 2 * F)]
NDUM = 34  # PE warm-up matmuls issued while input DMAs land
NFILL = 30  # PE keep-warm matmuls bridging the S-transition gap

PK1W = 2 * VF + 2 * CWW  # [ohf0 | ohf1 | cw]
BFW = LH + DM  # [one-hot select cols | rhs-table init image]

_PROG = None
LAST_RESULT = None  # BassKernelResults of the most recent run (for harnesses)


def _build_program():
    import concourse.mybir as mybir
    import concourse.tile as tile
    from concourse import bacc

    f32 = mybir.dt.float32
    bf16 = mybir.dt.bfloat16
    AO = mybir.AluOpType
    AF = mybir.ActivationFunctionType

    nc = bacc.Bacc("TRN2", target_bir_lowering=False, debug=False,
                   num_devices=NCORES)

    pk1 = nc.dram_tensor("pk1", [128, PK1W], bf16, kind="ExternalInput")
    pk2 = nc.dram_tensor("pk2", [128, V * D_STFT], bf16, kind="ExternalInput")
    nyw = nc.dram_tensor("nyw", [1, V * D_STFT], bf16, kind="ExternalInput")
    bfsel = nc.dram_tensor("bfsel", [KOUT, BFW], bf16, kind="ExternalInput")
    out = nc.dram_tensor("out", [LH, DM], bf16, kind="ExternalOutput")

    with tile.TileContext(nc) as tc:
        with (
            tc.tile_pool(name="consts", bufs=1) as cpool,
            tc.tile_pool(name="work", bufs=1) as wpool,
            tc.tile_pool(name="tmp", bufs=2) as tpool,
            tc.tile_pool(name="ostg", bufs=3) as ospool,
        ):
            # ---- const loads (issue order = need order) ---------------------
            PK1 = cpool.tile([128, PK1W], bf16, tag="pk1")
            nc.sync.dma_start(out=PK1[:], in_=pk1[:])
            PK2 = cpool.tile([128, V * D_STFT], bf16, tag="pk2")
            nc.sync.dma_start(out=PK2[:], in_=pk2[:])
            BF = cpool.tile([KOUT, BFW], bf16, tag="bf")
            nc.sync.dma_start(out=BF[:], in_=bfsel[:])
            NYW = cpool.tile([1, V * D_STFT], bf16, tag="nyw")
            nc.sync.dma_start(out=NYW[:], in_=nyw[:])

            OHF = [PK1[:, 0:VF], PK1[:, VF:2 * VF]]
            CW = PK1[:, 2 * VF:]
            WP = PK2
            WNR = NYW

            # ---- on-chip work tiles ----------------------------------------
            ZW = wpool.tile([128, 128], bf16, tag="zw")
            nc.vector.memset(ZW[:], 0.0)
            # tiny ACTIVATE(Copy) up front hoists its ACT table load into
            # the input-DMA dead zone (otherwise it lands right before the
            # first PSUM drain, on the output critical path)
            CPS = wpool.tile([1, 8], f32, tag="cps")
            nc.scalar.copy(out=CPS[:], in_=ZW[0:1, 0:8])
            # rhs table init: rows 0..64 [0 | bias], rows 65..80 [emb | 0]
            RT = wpool.tile([KOUT, DM], bf16, tag="rt")
            nc.vector.tensor_copy(out=RT[:], in_=BF[:, LH:])
            MAGH = wpool.tile([128, VF], bf16, tag="magh")
            NYB = wpool.tile([1, VF], bf16, tag="nyb")

            with tc.tile_pool(name="psum_s", bufs=1, space="PSUM") as psp:
                S = psp.tile([F, D_STFT], f32, tag="s")

                with (
                    tc.tile_pool(name="psum_dum", bufs=1, space="PSUM") as pdm,
                    tc.tile_pool(name="psum_re", bufs=2, space="PSUM") as pre,
                    tc.tile_pool(name="psum_im", bufs=2, space="PSUM") as pim,
                    tc.tile_pool(name="psum_ny", bufs=2, space="PSUM") as pny,
                ):
                    # PE warm-up: no input deps, scheduler runs these first;
                    # ~18 x (ldw+mm) ~= 3.4us busy -> HAM releases to 2.4 GHz
                    # right as the first real matmul's data lands.
                    DU = pdm.tile([128, 128], f32, tag="du")
                    for _ in range(NDUM):
                        nc.tensor.matmul(out=DU[:], lhsT=ZW[:], rhs=ZW[:],
                                         start=True, stop=True)

                    first_s = [True]

                    def proj_mm(lhsT, rhs, stop=False):
                        nc.tensor.matmul(out=S[:], lhsT=lhsT, rhs=rhs,
                                         start=first_s[0], stop=stop)
                        first_s[0] = False

                    for c0, cn in CHUNKS:
                        re = pre.tile([128, 7 * F], f32, tag="re")
                        im = pim.tile([128, 7 * F], f32, tag="im")
                        ny = pny.tile([1, 7 * F], f32, tag="ny")
                        for c in range(2):
                            cb = c * CWW
                            rhs = OHF[c][:, c0:c0 + cn]
                            nc.tensor.matmul(
                                out=re[:, :cn], lhsT=CW[:, cb:cb + 128],
                                rhs=rhs, start=(c == 0), stop=(c == 1))
                            nc.tensor.matmul(
                                out=im[:, :cn],
                                lhsT=CW[:, cb + NFREQ:cb + NFREQ + 128],
                                rhs=rhs, start=(c == 0), stop=(c == 1))
                            nc.tensor.matmul(
                                out=ny[:, :cn], lhsT=CW[:, cb + 128:cb + 129],
                                rhs=rhs, start=(c == 0), stop=(c == 1))
                        with tc.high_priority():
                            # |spec| = sqrt(re^2 + im^2); squares on ACT (DVE
                            # cannot read two PSUM operands)
                            t1 = tpool.tile([128, 7 * F], f32, tag="sq1")
                            t2 = tpool.tile([128, 7 * F], f32, tag="sq2")
                            nc.scalar.square(out=t1[:, :cn], in_=re[:, :cn])
                            nc.scalar.square(out=t2[:, :cn], in_=im[:, :cn])
                            nc.vector.tensor_tensor(
                                out=t1[:, :cn], in0=t1[:, :cn], in1=t2[:, :cn],
                                op=AO.add)
                            nc.scalar.sqrt(out=MAGH[:, c0:c0 + cn],
                                           in_=t1[:, :cn])
                            nc.scalar.activation(NYB[:, c0:c0 + cn],
                                                 ny[:, :cn], AF.Abs)
                            for v in range(c0 // F, (c0 + cn) // F):
                                proj_mm(MAGH[:, v * F:(v + 1) * F],
                                        WP[:, v * D_STFT:(v + 1) * D_STFT])

                    with tc.high_priority():
                        # nyquist bin: 16 tiny K=1 matmuls off the |ny| row
                        for v in range(V):
                            proj_mm(NYB[0:1, v * F:(v + 1) * F],
                                    WNR[0:1, v * D_STFT:(v + 1) * D_STFT],
                                    stop=(v == V - 1))

                    # keep-warm fillers: ready once chunk-2 |mag| exists, so
                    # the scheduler slots them into the PE idle window while
                    # the S tail (mag3/ny/RT-add) runs on ACT/DVE. Without
                    # these the ~3us gap re-throttles HAM and the whole
                    # output phase runs at 1.2 GHz.
                    for _ in range(NFILL):
                        nc.tensor.matmul(out=DU[:],
                                         lhsT=MAGH[:, 7 * F - 128:7 * F],
                                         rhs=ZW[:], start=True, stop=True)

                with tc.high_priority():
                    # rhs table rows 0..64 = S + bias image (in-place add)
                    nc.vector.tensor_tensor(
                        out=RT[0:F, D_EMB:DM], in0=S[:],
                        in1=RT[0:F, D_EMB:DM], op=AO.add)

            # ---- output: one fused matmul per 128-row tile ------------------
            with tc.tile_pool(name="psum_out", bufs=7, space="PSUM") as pout:
                for q in range(NQ):
                    os_ = ospool.tile([128, 4 * DM], bf16, tag="os")
                    for a in range(4):
                        ti = q * 4 + a
                        po = pout.tile([128, DM], f32, tag="po")
                        nc.tensor.matmul(
                            out=po[:], lhsT=BF[:, ti * 128:(ti + 1) * 128],
                            rhs=RT[0:KOUT, :], start=True, stop=True)
                        sl = os_[:, a * DM:(a + 1) * DM]
                        if a % 2 == 0:
                            nc.vector.tensor_copy(out=sl, in_=po[:])
                        else:
                            nc.scalar.copy(out=sl, in_=po[:])
                    nc.sync.dma_start(
                        out=out[q * 512:(q + 1) * 512, :]
                        .rearrange("(a p) e -> p a e", p=128),
                        in_=os_[:].rearrange("p (a e) -> p a e", a=4))

    nc.finalize()
    return nc


def _host_consts():
    import ml_dtypes

    bf16 = ml_dtypes.bfloat16
    n = np.arange(NFFT)
    window = 0.5 - 0.5 * np.cos(2.0 * np.pi * n / NFFT)
    k = np.arange(NFREQ)
    ang = 2.0 * np.pi * np.outer(n, k) / NFFT  # (256, 129)
    wcos = (window[:, None] * np.cos(ang)).astype(np.float32)
    wsin = (window[:, None] * np.sin(ang)).astype(np.float32)
    cw = np.zeros((128, 2 * CWW), np.float32)
    for c in range(2):
        rows = slice(c * 128, (c + 1) * 128)
        blk = np.zeros((128, CWW), np.float32)
        blk[:, :128] = wcos[rows, :128]
        blk[:, 128] = wcos[rows][:, 128]  # nyquist cos column
        blk[:, NFREQ:NFREQ + 128] = wsin[rows, :128]
        cw[:, c * CWW:(c + 1) * CWW] = blk
    return cw.astype(bf16)


def kernel(input_ids, emb_table, proj_w, proj_b):
    global _PROG, LAST_RESULT
    import ml_dtypes

    from concourse.bass_utils import run_bass_kernel_spmd

    bf16 = ml_dtypes.bfloat16
    ids = np.asarray(input_ids).astype(np.int64)
    emb = np.asarray(emb_table).astype(np.float32)
    pw = np.asarray(proj_w).astype(np.float32)
    pb = np.asarray(proj_b).astype(np.float32)

    cw = _host_consts()

    # pk1 cols [2*VF:] = cw; per-core ohf fills cols [:2*VF]
    # pk2: proj weights, rows k=0..127, cols v*154+o  (proj_w row i=k*V+v)
    pk2 = np.zeros((128, V * D_STFT), np.float32)
    for v in range(V):
        pk2[:, v * D_STFT:(v + 1) * D_STFT] = pw[np.arange(128) * V + v]
    pk2 = pk2.astype(bf16)

    # nyw: nyquist-bin proj weights as a single partition-0 row
    nywr = np.zeros((1, V * D_STFT), np.float32)
    for v in range(V):
        nywr[0, v * D_STFT:(v + 1) * D_STFT] = pw[128 * V + v]
    nywr = nywr.astype(bf16)

    # rhs-table init image: rows 0..64 [0 | bias], rows 65..80 [emb | 0]
    rtimg = np.zeros((KOUT, DM), np.float32)
    rtimg[:F, D_EMB:] = pb[None, :]
    rtimg[F:, :D_EMB] = emb

    vr = np.arange(V)
    in_maps = []
    for core in range(NCORES):
        b, h = divmod(core, 2)
        padded = np.pad(ids[b], 128, mode="reflect")
        seg = padded[LH * h:LH * h + 64 * (F - 1) + NFFT]  # (4352,)
        pk1 = np.zeros((128, PK1W), bf16)
        for c in range(2):
            sv = seg[(128 * c + np.arange(128))[:, None]
                     + 64 * np.arange(F)[None, :]]  # (128, F)
            oh = (sv[:, None, :] == vr[None, :, None])  # (128, V, F)
            pk1[:, c * VF:(c + 1) * VF] = oh.reshape(128, VF)
        pk1[:, 2 * VF:] = cw

        ids_h = ids[b, LH * h:LH * (h + 1)]
        t = np.arange(LH)
        floc = ((129 * (t + LH * h)) >> 13) - 64 * h
        bf = np.zeros((KOUT, BFW), np.float32)
        bf[floc, t] = 1.0
        bf[F + ids_h, t] = 1.0
        bf[:, LH:] = rtimg
        in_maps.append({
            "pk1": pk1, "pk2": pk2, "nyw": nywr,
            "bfsel": bf.astype(bf16),
        })

    if _PROG is None:
        _PROG = _build_program()

    res = run_bass_kernel_spmd(_PROG, in_maps, core_ids=list(range(NCORES)))
    LAST_RESULT = res

    full = np.zeros((B, L, DM), np.float32)
    for core in range(NCORES):
        b, h = divmod(core, 2)
        full[b, LH * h:LH * (h + 1), :] = \
            res.results[core]["out"].astype(np.float32)
    return full
